# revision 6
# baseline (speedup 1.0000x reference)
"""Multi-head attention (B=4, S=2048, D=1024, H=16, causal) on 8 trn2 cores.

Sharding: core c -> (batch b = c//2, head-group hg = c%2 of 8 heads).
Host pre-transposes/casts activations to bf16 [D, S] and slices weights;
device computes a partial [S, D] output (its head-group's contribution
through the output projection); host sums the pair per batch and adds bo.

v2 schedule:
- score matmuls for the two heads of a pair are interleaved so their
  64x128 PE row-tiles run concurrently;
- tiling-mode phases are batched (scores 64-mode, AV/proj 128-mode) to
  minimize PE array mode-switch drains;
- the softmax-denominator broadcast runs on gpsimd (partition_broadcast)
  instead of a K=1 matmul;
- bv is folded into the v projection (attn rows sum to 1), so finalize is
  a single vector multiply per head;
- projections/output-projection are chopped into ~2-matmul background
  items popped between attention stages under a per-round PE budget,
  with force-drain points preserving PE-queue dependency order.
"""

import numpy as np
import ml_dtypes

import concourse.bacc as bacc
import concourse.bass as bass
import concourse.mybir as mybir
import concourse.tile as tile
from concourse.bass_utils import run_bass_kernel_spmd

B, S, D, H = 4, 2048, 1024, 16
DH = D // H          # 64
HG = H // 2          # 8 heads per core
DG = HG * DH         # 512 dims per core
N_CORES = 8

BF16 = mybir.dt.bfloat16
F32 = mybir.dt.float32

ST = S // 128        # 16 seq tiles of 128
QB = S // 512        # 4 query blocks of 512
KT = D // 128        # 8 contraction tiles for the input projections
VBLK = DH + 1        # 65: per-head v columns + ones column
AF = mybir.ActivationFunctionType
ALU = mybir.AluOpType


def build_program(loop_r=0):
    """loop_r > 0 builds a measurement variant that repeats the whole body
    loop_r times inside an on-device loop (for timing via slope)."""
    nc = bacc.Bacc("TRN2", target_bir_lowering=False, debug=False,
                   num_devices=N_CORES)

    xq = nc.declare_dram_parameter("xq", [D, S], BF16, isOutput=False)
    xk = nc.declare_dram_parameter("xk", [D, S], BF16, isOutput=False)
    xv = nc.declare_dram_parameter("xv", [D, S], BF16, isOutput=False)
    wq = nc.declare_dram_parameter("wq", [D, DG], BF16, isOutput=False)
    wk = nc.declare_dram_parameter("wk", [D, DG], BF16, isOutput=False)
    wv = nc.declare_dram_parameter("wv", [D, DG], BF16, isOutput=False)
    wo = nc.declare_dram_parameter("wo", [DG, D], BF16, isOutput=False)
    bq = nc.declare_dram_parameter("bq", [DG, 1], F32, isOutput=False)
    bk = nc.declare_dram_parameter("bk", [DG, 1], F32, isOutput=False)
    bv = nc.declare_dram_parameter("bv", [DG, 1], F32, isOutput=False)
    out = nc.declare_dram_parameter("out", [S, D], F32, isOutput=True)

    with tile.TileContext(nc) as tc:
        with (
            tc.tile_pool(name="persist", bufs=1) as persist,
            tc.tile_pool(name="xin", bufs=4) as xin,
            tc.tile_pool(name="xvin", bufs=1) as xvin,
            tc.tile_pool(name="exp", bufs=6) as expp,
            tc.tile_pool(name="small", bufs=4) as small,
            tc.tile_pool(name="rbp", bufs=4) as rbp,
            tc.tile_pool(name="outp", bufs=2) as outp,
            tc.tile_pool(name="ps512", bufs=2, space="PSUM") as ps512,
            tc.tile_pool(name="pssc", bufs=2, space="PSUM") as pssc,
            tc.tile_pool(name="psav", bufs=2, space="PSUM") as psav,
        ):
            import contextlib
            loop_cm = tc.For_i(0, loop_r, 1) if loop_r else contextlib.nullcontext()
            with loop_cm:
                emit_body(nc, tc, locals())
    nc.compile()
    return nc


def emit_body(nc, tc, pools):
    persist = pools["persist"]; xin = pools["xin"]; xvin = pools["xvin"]
    expp = pools["expp"]; small = pools["small"]; rbp = pools["rbp"]
    outp = pools["outp"]
    ps512 = pools["ps512"]; pssc = pools["pssc"]; psav = pools["psav"]
    xq = pools["xq"]; xk = pools["xk"]; xv = pools["xv"]
    wq = pools["wq"]; wk = pools["wk"]; wv = pools["wv"]; wo = pools["wo"]
    bq = pools["bq"]; bk = pools["bk"]; bv = pools["bv"]; out = pools["out"]

    # ---- resident weights / constants (k/q-proj inputs first) ----
    wq_sb = persist.tile([128, KT * DG], BF16, tag="wq")
    wk_sb = persist.tile([128, KT * DG], BF16, tag="wk")
    wv_sb = persist.tile([128, KT * DG], BF16, tag="wv")
    wo_sb = persist.tile([128, 4 * D], BF16, tag="wo")
    bq_sb = persist.tile([128, 4], F32, tag="bq")
    bk_sb = persist.tile([128, 4], F32, tag="bk")
    bv_row = persist.tile([1, DG], F32, tag="bvrow")
    bv_bc = persist.tile([128, DG], F32, tag="bvbc")
    nc.sync.dma_start(
        wk_sb[:].rearrange("p (j c) -> p j c", j=KT),
        wk[:].rearrange("(j p) c -> p j c", p=128),
    )
    nc.sync.dma_start(
        bk_sb[:].rearrange("p (t o) -> p t o", o=1),
        bk[:].rearrange("(t p) o -> p t o", p=128),
    )
    nc.sync.dma_start(
        wq_sb[:].rearrange("p (j c) -> p j c", j=KT),
        wq[:].rearrange("(j p) c -> p j c", p=128),
    )
    nc.sync.dma_start(
        bq_sb[:].rearrange("p (t o) -> p t o", o=1),
        bq[:].rearrange("(t p) o -> p t o", p=128),
    )

    # two tril mask tiles side by side: mask[p, f] = 1.0 if p <= f%128
    masks = persist.tile([128, 256], BF16, tag="masks")
    nc.gpsimd.memset(masks[:], 1.0)
    for mi in range(2):
        nc.gpsimd.affine_select(
            out=masks[:, bass.ts(mi, 128)],
            in_=masks[:, bass.ts(mi, 128)],
            compare_op=ALU.is_ge,
            fill=0.0,
            base=0,
            pattern=[[1, 128]],
            channel_multiplier=-1,
        )

    # bv broadcast row: folded into the v projection (attn rows sum to 1)
    nc.sync.dma_start(
        bv_row[:].rearrange("o (f one) -> o f one", one=1),
        bv[:].rearrange("(o f) one -> o f one", o=1),
    )
    nc.gpsimd.partition_broadcast(bv_bc[:], bv_row[:], channels=128)

    # persistent activations
    qt = [persist.tile([128, S], BF16, tag=f"qt{t}", name=f"qt{t}") for t in range(4)]
    kt = [persist.tile([128, S], BF16, tag=f"kt{t}", name=f"kt{t}") for t in range(4)]
    v_sb = persist.tile([128, ST * HG * VBLK], BF16, tag="v_sb")
    ao = [persist.tile([128, S], BF16, tag=f"ao{t}", name=f"ao{t}") for t in range(4)]

    # ones columns of v blocks (written before the v adds below)
    v_view = v_sb[:].rearrange("p (s h c) -> p s h c", s=ST, h=HG, c=VBLK)
    nc.gpsimd.memset(v_view[:, :, :, DH : DH + 1], 1.0)

    # input chunk prefetch (one DMA per tensor per 512-block)
    chunk_tiles = {}

    def prefetch_chunk(n):
        xk_sb = xin.tile([128, KT * 512], BF16, tag="xkq", name=f"xk_sb{n}")
        nc.sync.dma_start(
            xk_sb[:].rearrange("p (j c) -> p j c", j=KT),
            xk[:, bass.ts(n, 512)].rearrange("(j p) c -> p j c", p=128),
        )
        xq_sb = xin.tile([128, KT * 512], BF16, tag="xkq", name=f"xq_sb{n}")
        nc.sync.dma_start(
            xq_sb[:].rearrange("p (j c) -> p j c", j=KT),
            xq[:, bass.ts(n, 512)].rearrange("(j p) c -> p j c", p=128),
        )
        chunk_tiles[n] = (xk_sb, xq_sb)

    prefetch_chunk(0)
    nc.sync.dma_start(
        wv_sb[:].rearrange("p (j c) -> p j c", j=KT),
        wv[:].rearrange("(j p) c -> p j c", p=128),
    )
    xv_sb = xvin.tile([128, KT * S], BF16, tag="xv")
    nc.sync.dma_start(
        xv_sb[:].rearrange("p (j c) -> p j c", j=KT),
        xv[:].rearrange("(j p) c -> p j c", p=128),
    )
    xv_t = [xv_sb[:, bass.ts(j, S)] for j in range(KT)]
    nc.sync.dma_start(
        wo_sb[:].rearrange("p (j c) -> p j c", j=4),
        wo[:].rearrange("(j p) c -> p j c", p=128),
    )

    # ---------------- background (bg) work machinery ----------------
    # Items are (pe_cost_ns, fn). The queue order is a dependency-safe
    # global order; ensure() force-drains up to a tag before the rounds
    # that consume the produced tiles.
    bg = []          # list of (cost, fn)
    tag_pos = {}     # tag -> index in bg after which the tag is satisfied
    proj_state = {}

    def bg_push(cost, fn):
        bg.append((cost, fn))

    def bg_mark(tag):
        tag_pos[tag] = len(bg)

    bg_idx = [0]

    def bg_pop(budget):
        while bg_idx[0] < len(bg) and budget > 0:
            cost, fn = bg[bg_idx[0]]
            bg_idx[0] += 1
            fn()
            budget -= cost

    def bg_ensure(tag):
        target = tag_pos.get(tag, 0)
        while bg_idx[0] < target:
            cost, fn = bg[bg_idx[0]]
            bg_idx[0] += 1
            fn()

    # ---- projection emitters, chopped into 2-MM items ----
    def kproj_mm(n, t, ph):
        def fn():
            xk_sb, _ = chunk_tiles[n]
            if ph == 0:
                proj_state[("k", n, t)] = ps512.tile(
                    [128, 512], F32, tag="mm512", name="psk")
            ps = proj_state[("k", n, t)]
            for j in (2 * ph, 2 * ph + 1):
                nc.tensor.matmul(
                    ps[:],
                    wk_sb[:, j * DG + t * 128 : j * DG + (t + 1) * 128],
                    xk_sb[:, bass.ts(j, 512)],
                    start=(j == 0),
                    stop=(j == KT - 1),
                )
            if ph == 3:
                nc.vector.tensor_scalar_add(
                    kt[t][:, bass.ts(n, 512)], ps[:], bk_sb[:, t : t + 1]
                )
        return fn

    def qproj_mm(n, t, ph):
        def fn():
            _, xq_sb = chunk_tiles[n]
            if ph == 0:
                proj_state[("q", n, t)] = ps512.tile(
                    [128, 512], F32, tag="mm512", name="psq")
            ps = proj_state[("q", n, t)]
            for j in (2 * ph, 2 * ph + 1):
                nc.tensor.matmul(
                    ps[:],
                    wq_sb[:, j * DG + t * 128 : j * DG + (t + 1) * 128],
                    xq_sb[:, bass.ts(j, 512)],
                    start=(j == 0),
                    stop=(j == KT - 1),
                )
            if ph == 3:
                nc.vector.tensor_scalar(
                    qt[t][:, bass.ts(n, 512)], ps[:],
                    bq_sb[:, t : t + 1], 0.125, ALU.add, ALU.mult,
                )
        return fn

    def vproj_mm(s, ph):
        def fn():
            if ph == 0:
                proj_state[("v", s)] = ps512.tile(
                    [128, 512], F32, tag="mm512", name="psv")
            ps = proj_state[("v", s)]
            for j in (2 * ph, 2 * ph + 1):
                nc.tensor.matmul(
                    ps[:],
                    xv_t[j][:, bass.ts(s, 128)],
                    wv_sb[:, bass.ts(j, DG)],
                    start=(j == 0),
                    stop=(j == KT - 1),
                )
            if ph == 3:
                nc.vector.tensor_add(
                    v_view[:, s, :, 0:DH],
                    ps[:].rearrange("p (h c) -> p h c", c=DH),
                    bv_bc[:].rearrange("p (h c) -> p h c", c=DH),
                )
        return fn

    def oproj_mm(s, m, ph):
        def fn():
            if ph == 0 and m == 0:
                proj_state[("ob", s)] = outp.tile(
                    [128, 1024], F32, tag="ob", name="ob")
            if ph == 0:
                proj_state[("po", s, m)] = ps512.tile(
                    [128, 512], F32, tag="mm512", name="po")
            po = proj_state[("po", s, m)]
            for kk in (2 * ph, 2 * ph + 1):
                nc.tensor.matmul(
                    po[:],
                    ao[kk][:, bass.ts(s, 128)],
                    wo_sb[:, kk * D + m * 512 : kk * D + (m + 1) * 512],
                    start=(kk == 0),
                    stop=(kk == 3),
                )
            if ph == 1:
                ob = proj_state[("ob", s)]
                nc.vector.tensor_copy(ob[:, bass.ts(m, 512)], po[:])
                if m == 1:
                    nc.sync.dma_start(out[bass.ts(s, 128), :], ob[:])
        return fn

    def push_kq(n, t):
        for ph in range(4):
            bg_push(440, qproj_mm(n, t, ph))
        for ph in range(4):
            bg_push(440, kproj_mm(n, t, ph))

    def push_v4(n1):
        for s in range(4 * n1, 4 * n1 + 4):
            for ph in range(4):
                bg_push(440, vproj_mm(s, ph))

    def push_oproj(nblk):
        for s in range(4 * nblk, 4 * nblk + 4):
            for m in range(2):
                for ph in range(2):
                    bg_push(440, oproj_mm(s, m, ph))

    # Global dependency-safe bg order.
    # Block 0 prerequisites (q/k t=0 emitted directly before rounds).
    push_v4(0)
    bg_mark(("vband", 0))
    bg_push(0, lambda: prefetch_chunk(1))
    for t in range(1, 4):
        push_kq(0, t)
        bg_mark(("kq", 0, t))
    for n1 in range(1, QB):
        push_kq(n1, 0)
        bg_mark(("kq", n1, 0))
        push_v4(n1)
        bg_mark(("vband", n1))
        if n1 + 1 < QB:
            bg_push(0, lambda n1=n1: prefetch_chunk(n1 + 1))
        for t in range(1, 4):
            push_kq(n1, t)
            bg_mark(("kq", n1, t))
        if n1 >= 2:
            push_oproj(n1 - 2)
    # block QB-2's oproj rides the bg queue during block QB-1's rounds
    # (emitted after block QB-2's last finalize in program order).
    push_oproj(QB - 2)

    # ---------------- attention ----------------
    # Direct head work for (n=0, hp=0): q/k proj t=0.
    for ph in range(4):
        qproj_mm(0, 0, ph)()
    for ph in range(4):
        kproj_mm(0, 0, ph)()

    for n in range(QB):
        nk = 4 * (n + 1)
        for hp in range(0, HG, 2):
            t, hA, hB = hp // 2, hp, hp + 1
            bg_ensure(("kq", n, t))
            qA = qt[t][0:DH, bass.ts(n, 512)]
            qB = qt[t][DH:128, bass.ts(n, 512)]
            avA = psav.tile([VBLK, 512], F32, tag="av", name="avA")
            avB = psav.tile([VBLK, 512], F32, tag="av", name="avB")

            # unit u covers key tiles (2u, 2u+1) [full] or the diagonal
            # band pair (4n+2u', 4n+2u'+1) for the last two units.
            U = 2 * n + 2

            def stage1(u):
                scA = pssc.tile([128, 1024], F32, tag="sc", name="scA")
                scB = pssc.tile([128, 1024], F32, tag="sc", name="scB")
                if u < 2 * n:
                    j0 = 2 * u
                    for d in range(2):
                        for sc, q_ap, r in ((scA, qA, 0), (scB, qB, 1)):
                            nc.tensor.matmul(
                                sc[:, bass.ts(d, 512)],
                                kt[t][r * DH : (r + 1) * DH,
                                      bass.ts(j0 + d, 128)],
                                q_ap,
                                start=True,
                                stop=True,
                            )
                    exA = expp.tile([128, 1024], BF16, tag="ex", name="exA")
                    exB = expp.tile([128, 1024], BF16, tag="ex", name="exB")
                    nc.scalar.activation(exA[:], scA[:], AF.Exp)
                    nc.scalar.activation(exB[:], scB[:], AF.Exp)
                    return (exA, exB, None)
                rp = u - 2 * n
                r0, r1 = 2 * rp, 2 * rp + 1
                nw0, nw1 = 512 - 128 * r0, 512 - 128 * r1
                for ri, off, nw in ((r0, 0, nw0), (r1, nw0, nw1)):
                    for sc, q_ap, r in ((scA, qA, 0), (scB, qB, 1)):
                        nc.tensor.matmul(
                            sc[:, off : off + nw],
                            kt[t][r * DH : (r + 1) * DH,
                                  bass.ts(4 * n + ri, 128)],
                            q_ap[:, 128 * ri : 512],
                            start=True,
                            stop=True,
                        )
                exA = expp.tile([128, 1024], BF16, tag="ex", name="exbA")
                exB = expp.tile([128, 1024], BF16, tag="ex", name="exbB")
                nc.scalar.activation(exA[:, 0 : nw0 + nw1],
                                     scA[:, 0 : nw0 + nw1], AF.Exp)
                nc.scalar.activation(exB[:, 0 : nw0 + nw1],
                                     scB[:, 0 : nw0 + nw1], AF.Exp)
                for ex in (exA, exB):
                    nc.gpsimd.tensor_mul(
                        ex[:, 0:128], ex[:, 0:128], masks[:, 0:128])
                    nc.gpsimd.tensor_mul(
                        ex[:, nw0 : nw0 + 128], ex[:, nw0 : nw0 + 128],
                        masks[:, 128:256])
                return (exA, exB, (r0, r1, nw0, nw1))

            def stage2(u, exs):
                exA, exB, band = exs
                if band is None:
                    j0 = 2 * u
                    for av, ex, h in ((avA, exA, hA), (avB, exB, hB)):
                        for d in range(2):
                            j = j0 + d
                            nc.tensor.matmul(
                                av[:],
                                v_sb[:, (j * HG * VBLK + h * VBLK)
                                     : (j * HG * VBLK + h * VBLK) + VBLK],
                                ex[:, bass.ts(d, 512)],
                                start=(j == 0),
                                stop=False,
                            )
                    return
                r0, r1, nw0, nw1 = band
                for av, ex, h in ((avA, exA, hA), (avB, exB, hB)):
                    for ri, off, nw in ((r0, 0, nw0), (r1, nw0, nw1)):
                        j = 4 * n + ri
                        nc.tensor.matmul(
                            av[:, 128 * ri : 512],
                            v_sb[:, (j * HG * VBLK + h * VBLK)
                                 : (j * HG * VBLK + h * VBLK) + VBLK],
                            ex[:, off : off + nw],
                            start=(j == 0),
                            stop=(j == nk - 1),
                        )

            # per-round PE slack under the ACT exp time, in ns
            def budget(u):
                if u < 2 * n:
                    return 800
                return 700 if u == 2 * n else 500

            prev = None
            for u in range(U):
                exs = stage1(u)
                if prev is not None:
                    bg_pop(budget(u - 1))
                    if prev[0] == 2 * n:
                        # band stage2 needs this block's v tiles
                        bg_ensure(("vband", n))
                    stage2(*prev)
                else:
                    bg_pop(1300)
                prev = (u, exs)
            bg_pop(budget(U - 1))
            if prev[0] == 2 * n:
                bg_ensure(("vband", n))
            stage2(*prev)

            # finalize: recip of the ones-row, gpsimd broadcast, one mul
            recipA = small.tile([1, 512], F32, tag="recip", name="recipA")
            nc.vector.reciprocal(recipA[:], avA[DH : DH + 1, :])
            recipB = small.tile([1, 512], F32, tag="recip", name="recipB")
            nc.vector.reciprocal(recipB[:], avB[DH : DH + 1, :])
            rbA = rbp.tile([DH, 512], F32, tag="rb", name="rbA")
            nc.gpsimd.partition_broadcast(rbA[:], recipA[:], channels=DH)
            rbB = rbp.tile([DH, 512], F32, tag="rb", name="rbB")
            nc.gpsimd.partition_broadcast(rbB[:], recipB[:], channels=DH)
            bg_pop(500)
            nc.vector.tensor_mul(
                ao[t][0:DH, bass.ts(n, 512)], avA[0:DH, :], rbA[:])
            nc.vector.tensor_mul(
                ao[t][DH:128, bass.ts(n, 512)], avB[0:DH, :], rbB[:])
            bg_pop(500)

    # drain remaining bg (oproj of block QB-2 stragglers etc.)
    bg_pop(float("inf"))
    # output projection of the last block
    for s in range(4 * (QB - 1), 4 * (QB - 1) + 4):
        for m in range(2):
            for ph in range(2):
                oproj_mm(s, m, ph)()


_NC = None


def _get_program():
    global _NC
    if _NC is None:
        _NC = build_program()
    return _NC


def make_in_maps(query, key, value, Wq, bq, Wk, bk, Wv, bv, Wo):
    bf = ml_dtypes.bfloat16
    in_maps = []
    xqs = [np.ascontiguousarray(query[b].T).astype(bf) for b in range(B)]
    xks = [np.ascontiguousarray(key[b].T).astype(bf) for b in range(B)]
    xvs = [np.ascontiguousarray(value[b].T).astype(bf) for b in range(B)]
    for c in range(N_CORES):
        b, hg = c // 2, c % 2
        sl = slice(hg * DG, (hg + 1) * DG)
        in_maps.append({
            "xq": xqs[b], "xk": xks[b], "xv": xvs[b],
            "wq": np.ascontiguousarray(Wq[sl, :].T).astype(bf),
            "wk": np.ascontiguousarray(Wk[sl, :].T).astype(bf),
            "wv": np.ascontiguousarray(Wv[sl, :].T).astype(bf),
            "wo": np.ascontiguousarray(Wo[:, sl].T).astype(bf),
            "bq": np.asarray(bq[sl], np.float32).reshape(DG, 1),
            "bk": np.asarray(bk[sl], np.float32).reshape(DG, 1),
            "bv": np.asarray(bv[sl], np.float32).reshape(DG, 1),
        })
    return in_maps


def combine_outputs(results, bo):
    out = np.empty((B, S, D), np.float32)
    for b in range(B):
        out[b] = results[2 * b]["out"] + results[2 * b + 1]["out"]
        out[b] += np.asarray(bo, np.float32)[None, :]
    return out


def kernel(query, key, value, mask, Wq, bq, Wk, bk, Wv, bv, Wo, bo):
    # mask is the causal tril mask from the reference problem; causality is
    # implemented directly in the device kernel.
    nc = _get_program()
    in_maps = make_in_maps(
        np.asarray(query, np.float32), np.asarray(key, np.float32),
        np.asarray(value, np.float32), np.asarray(Wq, np.float32),
        np.asarray(bq, np.float32), np.asarray(Wk, np.float32),
        np.asarray(bk, np.float32), np.asarray(Wv, np.float32),
        np.asarray(bv, np.float32), np.asarray(Wo, np.float32),
    )
    res = run_bass_kernel_spmd(nc, in_maps, list(range(N_CORES)))
    return combine_outputs(res.results, np.asarray(bo, np.float32))


# revision 13
# speedup vs baseline: 1.3242x; 1.3242x over previous
"""Multi-head attention (B=4, S=2048, D=1024, H=16, causal) on 8 trn2 cores.

Sharding: core c -> (batch b = c//2, head-group hg = c%2 of 8 heads).
Host pre-transposes/casts activations to bf16 [D, S] and slices weights;
device computes a partial [S, D] output (its head-group's contribution
through the output projection); host sums the pair per batch and adds bo.

v2 schedule:
- score matmuls for the two heads of a pair are interleaved so their
  64x128 PE row-tiles run concurrently;
- tiling-mode phases are batched (scores 64-mode, AV/proj 128-mode) to
  minimize PE array mode-switch drains;
- the softmax-denominator broadcast runs on gpsimd (partition_broadcast)
  instead of a K=1 matmul;
- bv is folded into the v projection (attn rows sum to 1), so finalize is
  a single vector multiply per head;
- projections/output-projection are chopped into ~2-matmul background
  items popped between attention stages under a per-round PE budget,
  with force-drain points preserving PE-queue dependency order.
"""

import numpy as np
import ml_dtypes

import concourse.bacc as bacc
import concourse.bass as bass
import concourse.mybir as mybir
import concourse.tile as tile
from concourse.bass_utils import run_bass_kernel_spmd

B, S, D, H = 4, 2048, 1024, 16
DH = D // H          # 64
HG = H // 2          # 8 heads per core
DG = HG * DH         # 512 dims per core
N_CORES = 8

BF16 = mybir.dt.bfloat16
F32 = mybir.dt.float32

ST = S // 128        # 16 seq tiles of 128
QB = S // 512        # 4 query blocks of 512
KT = D // 128        # 8 contraction tiles for the input projections
VBLK = DH + 1        # 65: per-head v columns + ones column
AF = mybir.ActivationFunctionType
ALU = mybir.AluOpType


def build_program(loop_r=0):
    """loop_r > 0 builds a measurement variant that repeats the whole body
    loop_r times inside an on-device loop (for timing via slope)."""
    nc = bacc.Bacc("TRN2", target_bir_lowering=False, debug=False,
                   num_devices=N_CORES)

    xq = nc.declare_dram_parameter("xq", [D, S], BF16, isOutput=False)
    xk = nc.declare_dram_parameter("xk", [D, S], BF16, isOutput=False)
    xv = nc.declare_dram_parameter("xv", [D, S], BF16, isOutput=False)
    wq = nc.declare_dram_parameter("wq", [D, DG], BF16, isOutput=False)
    wk = nc.declare_dram_parameter("wk", [D, DG], BF16, isOutput=False)
    wv = nc.declare_dram_parameter("wv", [D, DG], BF16, isOutput=False)
    wo = nc.declare_dram_parameter("wo", [DG, D], BF16, isOutput=False)
    bq = nc.declare_dram_parameter("bq", [DG, 1], F32, isOutput=False)
    bk = nc.declare_dram_parameter("bk", [DG, 1], F32, isOutput=False)
    bv = nc.declare_dram_parameter("bv", [DG, 1], F32, isOutput=False)
    out = nc.declare_dram_parameter("out", [S, D], BF16, isOutput=True)

    with tile.TileContext(nc) as tc:
        with (
            tc.tile_pool(name="persist", bufs=1) as persist,
            tc.tile_pool(name="xin", bufs=4) as xin,
            tc.tile_pool(name="xvin", bufs=1) as xvin,
            tc.tile_pool(name="exp", bufs=6) as expp,
            tc.tile_pool(name="small", bufs=4) as small,
            tc.tile_pool(name="rbp", bufs=4) as rbp,
            tc.tile_pool(name="outp", bufs=2) as outp,
            tc.tile_pool(name="ps512", bufs=2, space="PSUM") as ps512,
            tc.tile_pool(name="pssc", bufs=2, space="PSUM") as pssc,
            tc.tile_pool(name="psav", bufs=2, space="PSUM") as psav,
        ):
            import contextlib
            loop_cm = tc.For_i(0, loop_r, 1) if loop_r else contextlib.nullcontext()
            with loop_cm:
                emit_body(nc, tc, locals())
    nc.compile()
    return nc


def emit_body(nc, tc, pools):
    persist = pools["persist"]; xin = pools["xin"]; xvin = pools["xvin"]
    expp = pools["expp"]; small = pools["small"]; rbp = pools["rbp"]
    outp = pools["outp"]
    ps512 = pools["ps512"]; pssc = pools["pssc"]; psav = pools["psav"]
    xq = pools["xq"]; xk = pools["xk"]; xv = pools["xv"]
    wq = pools["wq"]; wk = pools["wk"]; wv = pools["wv"]; wo = pools["wo"]
    bq = pools["bq"]; bk = pools["bk"]; bv = pools["bv"]; out = pools["out"]

    # ---- resident weights / constants (k/q-proj inputs first) ----
    wq_sb = persist.tile([128, KT * DG], BF16, tag="wq")
    wk_sb = persist.tile([128, KT * DG], BF16, tag="wk")
    wv_sb = persist.tile([128, KT * DG], BF16, tag="wv")
    wo_sb = persist.tile([128, 4 * D], BF16, tag="wo")
    bq_sb = persist.tile([128, 4], F32, tag="bq")
    bk_sb = persist.tile([128, 4], F32, tag="bk")
    bv_row = persist.tile([1, DG], F32, tag="bvrow")
    bv_bc = persist.tile([128, DG], F32, tag="bvbc")
    nc.sync.dma_start(
        wk_sb[:].rearrange("p (j c) -> p j c", j=KT),
        wk[:].rearrange("(j p) c -> p j c", p=128),
    )
    nc.sync.dma_start(
        bk_sb[:].rearrange("p (t o) -> p t o", o=1),
        bk[:].rearrange("(t p) o -> p t o", p=128),
    )
    nc.sync.dma_start(
        wq_sb[:].rearrange("p (j c) -> p j c", j=KT),
        wq[:].rearrange("(j p) c -> p j c", p=128),
    )
    nc.sync.dma_start(
        bq_sb[:].rearrange("p (t o) -> p t o", o=1),
        bq[:].rearrange("(t p) o -> p t o", p=128),
    )

    # two tril mask tiles side by side: mask[p, f] = 1.0 if p <= f%128
    masks = persist.tile([128, 256], BF16, tag="masks")
    nc.gpsimd.memset(masks[:], 1.0)
    for mi in range(2):
        nc.gpsimd.affine_select(
            out=masks[:, bass.ts(mi, 128)],
            in_=masks[:, bass.ts(mi, 128)],
            compare_op=ALU.is_ge,
            fill=0.0,
            base=0,
            pattern=[[1, 128]],
            channel_multiplier=-1,
        )

    # bv broadcast row: folded into the v projection (attn rows sum to 1)
    nc.sync.dma_start(
        bv_row[:].rearrange("o (f one) -> o f one", one=1),
        bv[:].rearrange("(o f) one -> o f one", o=1),
    )
    nc.gpsimd.partition_broadcast(bv_bc[:], bv_row[:], channels=128)

    # selector weights for the reciprocal partition-broadcast matmul:
    # bc[m, q] = sum_k sel[k, m] * recAB[k, q] with recAB rows 0/1 holding
    # the two heads' reciprocals -> bc rows 0-63 = recipA, 64-127 = recipB.
    sel = persist.tile([128, 128], BF16, tag="sel")
    nc.gpsimd.memset(sel[:], 0.0)
    nc.gpsimd.memset(sel[0:1, 0:DH], 1.0)
    nc.gpsimd.memset(sel[64:65, DH:128], 1.0)
    recAB = persist.tile([128, 512], BF16, tag="recAB")
    nc.gpsimd.memset(recAB[:], 0.0)

    # persistent activations
    qt = [persist.tile([128, S], BF16, tag=f"qt{t}", name=f"qt{t}") for t in range(4)]
    kt = [persist.tile([128, S], BF16, tag=f"kt{t}", name=f"kt{t}") for t in range(4)]
    v_sb = persist.tile([128, ST * HG * VBLK], BF16, tag="v_sb")
    ao = [persist.tile([128, S], BF16, tag=f"ao{t}", name=f"ao{t}") for t in range(4)]

    # ones columns of v blocks (written before the v adds below)
    v_view = v_sb[:].rearrange("p (s h c) -> p s h c", s=ST, h=HG, c=VBLK)
    nc.gpsimd.memset(v_view[:, :, :, DH : DH + 1], 1.0)

    # input chunk prefetch (one DMA per tensor per 512-block)
    chunk_tiles = {}

    def prefetch_chunk(n):
        xk_sb = xin.tile([128, KT * 512], BF16, tag="xkq", name=f"xk_sb{n}")
        nc.sync.dma_start(
            xk_sb[:].rearrange("p (j c) -> p j c", j=KT),
            xk[:, bass.ts(n, 512)].rearrange("(j p) c -> p j c", p=128),
        )
        xq_sb = xin.tile([128, KT * 512], BF16, tag="xkq", name=f"xq_sb{n}")
        nc.sync.dma_start(
            xq_sb[:].rearrange("p (j c) -> p j c", j=KT),
            xq[:, bass.ts(n, 512)].rearrange("(j p) c -> p j c", p=128),
        )
        chunk_tiles[n] = (xk_sb, xq_sb)

    prefetch_chunk(0)
    nc.sync.dma_start(
        wv_sb[:].rearrange("p (j c) -> p j c", j=KT),
        wv[:].rearrange("(j p) c -> p j c", p=128),
    )
    xv_sb = xvin.tile([128, KT * S], BF16, tag="xv")
    for quarter in range(4):
        nc.sync.dma_start(
            xv_sb[:].rearrange("p (j c) -> p j c", j=KT)[
                :, :, bass.ts(quarter, 512)],
            xv[:, bass.ts(quarter, 512)].rearrange("(j p) c -> p j c", p=128),
        )
    xv_t = [xv_sb[:, bass.ts(j, S)] for j in range(KT)]
    nc.sync.dma_start(
        wo_sb[:].rearrange("p (j c) -> p j c", j=4),
        wo[:].rearrange("(j p) c -> p j c", p=128),
    )

    # ---------------- background (bg) work machinery ----------------
    # Items are (pe_cost_ns, fn). The queue order is a dependency-safe
    # global order; ensure() force-drains up to a tag before the rounds
    # that consume the produced tiles.
    bg = []          # list of (cost, fn)
    tag_pos = {}     # tag -> index in bg after which the tag is satisfied
    proj_state = {}

    def bg_push(cost, fn):
        bg.append((cost, fn))

    def bg_mark(tag):
        tag_pos[tag] = len(bg)

    bg_idx = [0]

    def bg_pop(budget):
        while bg_idx[0] < len(bg) and budget > 0:
            cost, fn = bg[bg_idx[0]]
            bg_idx[0] += 1
            fn()
            budget -= cost

    def bg_ensure(tag):
        target = tag_pos.get(tag, 0)
        while bg_idx[0] < target:
            cost, fn = bg[bg_idx[0]]
            bg_idx[0] += 1
            fn()

    # ---- projection emitters, chopped into 2-MM items ----
    def kproj_mm(n, t, ph):
        def fn():
            xk_sb, _ = chunk_tiles[n]
            if ph == 0:
                proj_state[("k", n, t)] = ps512.tile(
                    [128, 512], F32, tag="mm512", name="psk")
            ps = proj_state[("k", n, t)]
            for j in (2 * ph, 2 * ph + 1):
                nc.tensor.matmul(
                    ps[:],
                    wk_sb[:, j * DG + t * 128 : j * DG + (t + 1) * 128],
                    xk_sb[:, bass.ts(j, 512)],
                    start=(j == 0),
                    stop=(j == KT - 1),
                )
            if ph == 3:
                nc.vector.tensor_scalar_add(
                    kt[t][:, bass.ts(n, 512)], ps[:], bk_sb[:, t : t + 1]
                )
        return fn

    def qproj_mm(n, t, ph):
        def fn():
            _, xq_sb = chunk_tiles[n]
            if ph == 0:
                proj_state[("q", n, t)] = ps512.tile(
                    [128, 512], F32, tag="mm512", name="psq")
            ps = proj_state[("q", n, t)]
            for j in (2 * ph, 2 * ph + 1):
                nc.tensor.matmul(
                    ps[:],
                    wq_sb[:, j * DG + t * 128 : j * DG + (t + 1) * 128],
                    xq_sb[:, bass.ts(j, 512)],
                    start=(j == 0),
                    stop=(j == KT - 1),
                )
            if ph == 3:
                nc.vector.tensor_scalar(
                    qt[t][:, bass.ts(n, 512)], ps[:],
                    bq_sb[:, t : t + 1], 0.125, ALU.add, ALU.mult,
                )
        return fn

    def vproj_mm(s, ph):
        def fn():
            if ph == 0:
                proj_state[("v", s)] = ps512.tile(
                    [128, 512], F32, tag="mm512", name="psv")
            ps = proj_state[("v", s)]
            for j in (2 * ph, 2 * ph + 1):
                nc.tensor.matmul(
                    ps[:],
                    xv_t[j][:, bass.ts(s, 128)],
                    wv_sb[:, bass.ts(j, DG)],
                    start=(j == 0),
                    stop=(j == KT - 1),
                )
            if ph == 3:
                nc.vector.tensor_add(
                    v_view[:, s, :, 0:DH],
                    ps[:].rearrange("p (h c) -> p h c", c=DH),
                    bv_bc[:].rearrange("p (h c) -> p h c", c=DH),
                )
        return fn

    def oproj_mm(s, m, ph):
        def fn():
            if ph == 0 and m == 0:
                proj_state[("ob", s)] = outp.tile(
                    [128, 1024], BF16, tag="ob", name="ob")
            if ph == 0:
                proj_state[("po", s, m)] = ps512.tile(
                    [128, 512], F32, tag="mm512", name="po")
            po = proj_state[("po", s, m)]
            for kk in (2 * ph, 2 * ph + 1):
                nc.tensor.matmul(
                    po[:],
                    ao[kk][:, bass.ts(s, 128)],
                    wo_sb[:, kk * D + m * 512 : kk * D + (m + 1) * 512],
                    start=(kk == 0),
                    stop=(kk == 3),
                )
            if ph == 1:
                ob = proj_state[("ob", s)]
                nc.vector.tensor_copy(ob[:, bass.ts(m, 512)], po[:])
                if m == 1:
                    nc.sync.dma_start(out[bass.ts(s, 128), :], ob[:])
        return fn

    def push_kq(n, t):
        for ph in range(4):
            bg_push(440, qproj_mm(n, t, ph))
        for ph in range(4):
            bg_push(440, kproj_mm(n, t, ph))

    def push_v4(n1):
        for s in range(4 * n1, 4 * n1 + 4):
            for ph in range(4):
                bg_push(440, vproj_mm(s, ph))

    def push_oproj(nblk):
        for s in range(4 * nblk, 4 * nblk + 4):
            for m in range(2):
                for ph in range(2):
                    bg_push(440, oproj_mm(s, m, ph))

    # Global dependency-safe bg order.
    # Block 0 prerequisites (q/k t=0 emitted directly before rounds).
    push_v4(0)
    bg_mark(("vband", 0))
    bg_push(0, lambda: prefetch_chunk(1))
    for t in range(1, 4):
        push_kq(0, t)
        bg_mark(("kq", 0, t))
    for n1 in range(1, QB):
        push_kq(n1, 0)
        bg_mark(("kq", n1, 0))
        push_v4(n1)
        bg_mark(("vband", n1))
        if n1 + 1 < QB:
            bg_push(0, lambda n1=n1: prefetch_chunk(n1 + 1))
        for t in range(1, 4):
            push_kq(n1, t)
            bg_mark(("kq", n1, t))
        if n1 >= 2:
            push_oproj(n1 - 2)
    # block QB-2's oproj rides the bg queue during block QB-1's rounds
    # (emitted after block QB-2's last finalize in program order).
    push_oproj(QB - 2)

    # ---------------- attention ----------------
    # Direct head work for (n=0, hp=0): q/k proj t=0.
    for ph in range(4):
        qproj_mm(0, 0, ph)()
    for ph in range(4):
        kproj_mm(0, 0, ph)()

    for n in range(QB):
        nk = 4 * (n + 1)
        for hp in range(0, HG, 2):
            t, hA, hB = hp // 2, hp, hp + 1
            bg_ensure(("kq", n, t))
            qA = qt[t][0:DH, bass.ts(n, 512)]
            qB = qt[t][DH:128, bass.ts(n, 512)]
            avA = psav.tile([VBLK, 512], F32, tag="av", name="avA")
            avB = psav.tile([VBLK, 512], F32, tag="av", name="avB")

            # unit u covers key tiles (2u, 2u+1) [full] or the diagonal
            # band pair (4n+2u', 4n+2u'+1) for the last two units.
            U = 2 * n + 2

            def stage1(u):
                scA = pssc.tile([128, 1024], F32, tag="sc", name="scA")
                scB = pssc.tile([128, 1024], F32, tag="sc", name="scB")
                if u < 2 * n:
                    j0 = 2 * u
                    for d in range(2):
                        for sc, q_ap, r in ((scA, qA, 0), (scB, qB, 1)):
                            nc.tensor.matmul(
                                sc[:, bass.ts(d, 512)],
                                kt[t][r * DH : (r + 1) * DH,
                                      bass.ts(j0 + d, 128)],
                                q_ap,
                                start=True,
                                stop=True,
                            )
                    exA = expp.tile([128, 1024], BF16, tag="ex", name="exA")
                    exB = expp.tile([128, 1024], BF16, tag="ex", name="exB")
                    nc.scalar.activation(exA[:], scA[:], AF.Exp)
                    nc.scalar.activation(exB[:], scB[:], AF.Exp)
                    return (exA, exB, None)
                rp = u - 2 * n
                r0, r1 = 2 * rp, 2 * rp + 1
                nw0, nw1 = 512 - 128 * r0, 512 - 128 * r1
                for ri, off, nw in ((r0, 0, nw0), (r1, nw0, nw1)):
                    for sc, q_ap, r in ((scA, qA, 0), (scB, qB, 1)):
                        nc.tensor.matmul(
                            sc[:, off : off + nw],
                            kt[t][r * DH : (r + 1) * DH,
                                  bass.ts(4 * n + ri, 128)],
                            q_ap[:, 128 * ri : 512],
                            start=True,
                            stop=True,
                        )
                exA = expp.tile([128, 1024], BF16, tag="ex", name="exbA")
                exB = expp.tile([128, 1024], BF16, tag="ex", name="exbB")
                nc.scalar.activation(exA[:, 0 : nw0 + nw1],
                                     scA[:, 0 : nw0 + nw1], AF.Exp)
                nc.scalar.activation(exB[:, 0 : nw0 + nw1],
                                     scB[:, 0 : nw0 + nw1], AF.Exp)
                for ex in (exA, exB):
                    nc.vector.tensor_mul(
                        ex[:, 0:128], ex[:, 0:128], masks[:, 0:128])
                    nc.vector.tensor_mul(
                        ex[:, nw0 : nw0 + 128], ex[:, nw0 : nw0 + 128],
                        masks[:, 128:256])
                return (exA, exB, (r0, r1, nw0, nw1))

            def stage2(u, exs):
                exA, exB, band = exs
                if band is None:
                    j0 = 2 * u
                    for av, ex, h in ((avA, exA, hA), (avB, exB, hB)):
                        for d in range(2):
                            j = j0 + d
                            nc.tensor.matmul(
                                av[:],
                                v_sb[:, (j * HG * VBLK + h * VBLK)
                                     : (j * HG * VBLK + h * VBLK) + VBLK],
                                ex[:, bass.ts(d, 512)],
                                start=(j == 0),
                                stop=False,
                            )
                    return
                r0, r1, nw0, nw1 = band
                for av, ex, h in ((avA, exA, hA), (avB, exB, hB)):
                    for ri, off, nw in ((r0, 0, nw0), (r1, nw0, nw1)):
                        j = 4 * n + ri
                        nc.tensor.matmul(
                            av[:, 128 * ri : 512],
                            v_sb[:, (j * HG * VBLK + h * VBLK)
                                 : (j * HG * VBLK + h * VBLK) + VBLK],
                            ex[:, off : off + nw],
                            start=(j == 0),
                            stop=(j == nk - 1),
                        )

            # per-round PE slack under the ACT exp time, in ns
            def budget(u):
                if u < 2 * n:
                    return 800
                return 700 if u == 2 * n else 500

            prev = None
            for u in range(U):
                exs = stage1(u)
                if prev is not None:
                    bg_pop(budget(u - 1))
                    if prev[0] == 2 * n:
                        # band stage2 needs this block's v tiles
                        bg_ensure(("vband", n))
                    stage2(*prev)
                else:
                    bg_pop(1300)
                prev = (u, exs)
            bg_pop(budget(U - 1))
            if prev[0] == 2 * n:
                bg_ensure(("vband", n))
            stage2(*prev)

            # finalize: recips of the ones-rows (bf16), one selector matmul
            # broadcasting both across partitions, one evacuation copy, and
            # one normalize-multiply per head.
            with nc.allow_low_precision("softmax recip scale in bf16"):
                nc.vector.reciprocal(recAB[0:1, :], avA[DH : DH + 1, :])
                nc.vector.reciprocal(recAB[64:65, :], avB[DH : DH + 1, :])
            bc = ps512.tile([128, 512], F32, tag="mm512", name="bc")
            nc.tensor.matmul(bc[:], sel[:], recAB[:], start=True, stop=True)
            rbS = rbp.tile([128, 512], F32, tag="rb", name="rbS")
            nc.vector.tensor_copy(rbS[:], bc[:])
            bg_pop(500)
            nc.vector.tensor_mul(
                ao[t][0:DH, bass.ts(n, 512)], avA[0:DH, :], rbS[0:DH, :])
            nc.vector.tensor_mul(
                ao[t][DH:128, bass.ts(n, 512)], avB[0:DH, :], rbS[DH:128, :])
            bg_pop(500)

    # drain remaining bg (oproj of block QB-2 stragglers etc.)
    bg_pop(float("inf"))
    # output projection of the last block
    for s in range(4 * (QB - 1), 4 * (QB - 1) + 4):
        for m in range(2):
            for ph in range(2):
                oproj_mm(s, m, ph)()


_NC = None


def _get_program():
    global _NC
    if _NC is None:
        _NC = build_program()
    return _NC


def make_in_maps(query, key, value, Wq, bq, Wk, bk, Wv, bv, Wo):
    bf = ml_dtypes.bfloat16
    in_maps = []
    xqs = [np.ascontiguousarray(query[b].T).astype(bf) for b in range(B)]
    xks = [np.ascontiguousarray(key[b].T).astype(bf) for b in range(B)]
    xvs = [np.ascontiguousarray(value[b].T).astype(bf) for b in range(B)]
    for c in range(N_CORES):
        b, hg = c // 2, c % 2
        sl = slice(hg * DG, (hg + 1) * DG)
        in_maps.append({
            "xq": xqs[b], "xk": xks[b], "xv": xvs[b],
            "wq": np.ascontiguousarray(Wq[sl, :].T).astype(bf),
            "wk": np.ascontiguousarray(Wk[sl, :].T).astype(bf),
            "wv": np.ascontiguousarray(Wv[sl, :].T).astype(bf),
            "wo": np.ascontiguousarray(Wo[:, sl].T).astype(bf),
            "bq": np.asarray(bq[sl], np.float32).reshape(DG, 1),
            "bk": np.asarray(bk[sl], np.float32).reshape(DG, 1),
            "bv": np.asarray(bv[sl], np.float32).reshape(DG, 1),
        })
    return in_maps


def combine_outputs(results, bo):
    out = np.empty((B, S, D), np.float32)
    for b in range(B):
        out[b] = (np.asarray(results[2 * b]["out"], np.float32)
                  + np.asarray(results[2 * b + 1]["out"], np.float32))
        out[b] += np.asarray(bo, np.float32)[None, :]
    return out


def kernel(query, key, value, mask, Wq, bq, Wk, bk, Wv, bv, Wo, bo):
    # mask is the causal tril mask from the reference problem; causality is
    # implemented directly in the device kernel.
    nc = _get_program()
    in_maps = make_in_maps(
        np.asarray(query, np.float32), np.asarray(key, np.float32),
        np.asarray(value, np.float32), np.asarray(Wq, np.float32),
        np.asarray(bq, np.float32), np.asarray(Wk, np.float32),
        np.asarray(bk, np.float32), np.asarray(Wv, np.float32),
        np.asarray(bv, np.float32), np.asarray(Wo, np.float32),
    )
    res = run_bass_kernel_spmd(nc, in_maps, list(range(N_CORES)))
    return combine_outputs(res.results, np.asarray(bo, np.float32))
